# revision 1
# baseline (speedup 1.0000x reference)
"""Trainium2 Bass kernel for a 2-layer LSTM decoder (nn_Decoder).

Strategy: tensor-parallel over the hidden dimension across 8 NeuronCores.
Each core owns a 128-wide slice of H=1024 for both LSTM layers (its 512 of
the 4096 gate rows), and replicates the final fc layer so the autoregressive
input y needs no exchange.  The only cross-core traffic is an allgather of
each layer's hidden-state slice (128x64 bf16 = 16KB) per step, done with
direct SBUF->SBUF remote DMA (no HBM bounce, no ncfw collective floor).

v2 layout: gates are computed directly in feature-major orientation
out[gate_rows(128), batch(64)] with the weight tile as the stationary
operand and the (transposed) activations as the 64-row moving operand.
This halves the PE row count vs the moving-weight orientation (the PE
charges free-dim rows regardless of partition fill), and the gate chunks
land in PSUM exactly in the [hid, batch] layout the cell update wants --
no PE transpose, no PSUM->SBUF gate copy.

Gate rows are host-reordered to (f, i, o, g) so one fused Sigmoid over
[128, 192] covers f,i,o and one Tanh covers g.  Biases are folded into
the PSUM accumulation with K=1 matmuls (lhsT = bias row, rhs = ones row),
which is what makes the fused activations legal.  PSUM banks are split
(f,i,o | g per layer, fc halves) so an activation never reads a bank
that later matmuls of the same step still accumulate into.

The fc runs weight-stationary into two PSUM banks so relu of the first
256 output rows overlaps the matmuls of the last 256 (and layer-0's
x-part matmuls start after the first relu half).

Each exchange is ONE remote_dma_broadcast to the 7 peers (XOR-relative
routing; empirically the D2D lanes (delta bit 2 set) deliver to delta^2 so
those rdests are pre-compensated).  The destination slot is sender-pid *
64 columns via a register access pattern, so every core runs an identical
program.  Descriptor generation is pre-staged one step ahead on the Q7;
only trigger_dma sits on the critical path.

Flow control: h buffers are double-buffered by step parity; the LSTM's own
data-dependency chain guarantees a sender cannot run 2 steps ahead of any
receiver, so no credit messages are needed.
"""

import sys

sys.path.insert(0, "/opt/trn_rl_repo")

import numpy as np

B = 64
H = 1024
OUT = 512
N_CORES = 8
HPC = H // N_CORES  # 128 hidden units per core
SOS_VALUE = -2.0

_CACHE = {}


def _build(seq):
    from concourse import bacc, bass, mybir

    dt = mybir.dt
    f32 = dt.float32
    bf16 = dt.bfloat16
    AF = mybir.ActivationFunctionType
    ALU = mybir.AluOpType

    nc = bacc.Bacc("TRN2", target_bir_lowering=False, debug=False,
                   num_devices=N_CORES)

    # ---- DRAM I/O (per-core shards prepared on host) ----
    w0x_d = nc.dram_tensor("w0x", [4 * 128, 512], bf16, kind="ExternalInput")
    w0h_d = nc.dram_tensor("w0h", [8 * 128, 512], bf16, kind="ExternalInput")
    w1_d = nc.dram_tensor("w1", [16 * 128, 512], bf16, kind="ExternalInput")
    wfc_d = nc.dram_tensor("wfc", [8 * 128, 128], bf16, kind="ExternalInput")
    b0_d = nc.dram_tensor("b0", [1, 512], bf16, kind="ExternalInput")
    b1_d = nc.dram_tensor("b1", [1, 512], bf16, kind="ExternalInput")
    bfc_d = nc.dram_tensor("bfc", [128, 1], f32, kind="ExternalInput")
    ones_d = nc.dram_tensor("ones", [1, 64], bf16, kind="ExternalInput")
    h0i_d = nc.dram_tensor("h0i", [1024, 64], bf16, kind="ExternalInput")
    h1i_d = nc.dram_tensor("h1i", [1024, 64], bf16, kind="ExternalInput")
    c0i_d = nc.dram_tensor("c0i", [128, 64], f32, kind="ExternalInput")
    c1i_d = nc.dram_tensor("c1i", [128, 64], f32, kind="ExternalInput")
    yi_d = nc.dram_tensor("yi", [1024, 64], bf16, kind="ExternalInput")
    out_d = nc.dram_tensor("out", [seq, 128, 256], bf16, kind="ExternalOutput")

    # ---- SBUF ----
    w0x = nc.alloc_sbuf_tensor("w0xs", [128, 4 * 512], bf16)
    w0h = nc.alloc_sbuf_tensor("w0hs", [128, 8 * 512], bf16)
    w1 = nc.alloc_sbuf_tensor("w1s", [128, 16 * 512], bf16)
    wfc = nc.alloc_sbuf_tensor("wfcs", [128, 8 * 128], bf16)
    b0 = nc.alloc_sbuf_tensor("b0s", [1, 512], bf16)
    b1 = nc.alloc_sbuf_tensor("b1s", [1, 512], bf16)
    bfc = nc.alloc_sbuf_tensor("bfcs", [128, 1], f32)
    ones = nc.alloc_sbuf_tensor("oness", [1, 64], bf16)
    h0T = [nc.alloc_sbuf_tensor(f"h0T{p}", [128, 512], bf16) for p in range(2)]
    h1T = [nc.alloc_sbuf_tensor(f"h1T{p}", [128, 512], bf16) for p in range(2)]
    xT = [nc.alloc_sbuf_tensor(f"xT{p}", [128, 512], bf16) for p in range(2)]
    # per-layer cell-state scratch (bf16 where DVE 2x mode applies)
    sbg = [nc.alloc_sbuf_tensor(f"sbg{l}", [128, 192], bf16) for l in range(2)]
    tg = [nc.alloc_sbuf_tensor(f"tg{l}", [128, 64], bf16) for l in range(2)]
    cst = [nc.alloc_sbuf_tensor(f"cst{l}", [128, 64], f32) for l in range(2)]
    th = [nc.alloc_sbuf_tensor(f"th{l}", [128, 64], bf16) for l in range(2)]
    tmp = [[nc.alloc_sbuf_tensor(f"tmp{l}_{j}", [128, 64],
                                 f32 if j == 0 else bf16)
            for j in range(2)] for l in range(2)]

    # ---- PSUM (single-buffered; bank-split so ACT never reads a bank
    # other matmuls of the same step still write) ----
    ps_g0a = nc.alloc_psum_tensor("ps_g0a", [128, 192], f32)  # f,i,o
    ps_g0b = nc.alloc_psum_tensor("ps_g0b", [128, 64], f32)   # g
    ps_g1a = nc.alloc_psum_tensor("ps_g1a", [128, 192], f32)
    ps_g1b = nc.alloc_psum_tensor("ps_g1b", [128, 64], f32)
    ps_y = nc.alloc_psum_tensor("ps_y", [128, 64], f32)  # own y chunk

    # ---- semaphores ----
    S = lambda n: nc.alloc_semaphore(n)
    init = S("init")
    pe_g0a, pe_g0b = S("pe_g0a"), S("pe_g0b")
    pe_g1a, pe_g1b = S("pe_g1a"), S("pe_g1b")
    pe_y = S("pe_y")
    act_s0, act_tg0, act_tc0 = S("act_s0"), S("act_tg0"), S("act_tc0")
    act_s1, act_tg1, act_tc1 = S("act_s1"), S("act_tg1"), S("act_tc1")
    act_y = S("act_y")
    dv_c0, dv_c1 = S("dv_c0"), S("dv_c1")
    dv_h0, dv_h1 = S("dv_h0"), S("dv_h1")
    prep = S("prep")
    ls0, ls1, lsy = S("ls0"), S("ls1"), S("lsy")
    rs_h0 = [S(f"rs_h0_{p}") for p in range(2)]
    rs_h1 = [S(f"rs_h1_{p}") for p in range(2)]
    rs_y = [S(f"rs_y_{p}") for p in range(2)]
    dsem = S("dsem")

    N_INIT_DMA = 13
    INIT_V = 16 * N_INIT_DMA

    def fills(t):
        # number of exchange rounds into buffer t%2 after step t's exchange
        return t // 2 + 1

    # weight tile (k-chunk k, gate/out chunk g) as the stationary lhsT
    def wt(sb, k, g):
        return sb.ap()[:, 512 * k + 128 * g:512 * k + 128 * (g + 1)]

    def wtx(k, g):
        return wt(w0x, k, g)

    def htile(sb, k):
        return sb.ap()[:, 64 * k:64 * (k + 1)]

    # psum region for gate q (0=f,1=i,2=o in the a-bank; 3=g in the b-bank)
    def greg(psa, psb, g):
        return psb.ap() if g == 3 else psa.ap()[:, 64 * g:64 * (g + 1)]

    with nc.Block() as block:

        @block.sync
        def _(eng):
            eng.dma_start(
                w0x.ap().rearrange("p (t n) -> p t n", t=4),
                w0x_d.ap().rearrange("(t p) n -> p t n", p=128)).then_inc(init, 16)
            eng.dma_start(
                w0h.ap().rearrange("p (t n) -> p t n", t=8),
                w0h_d.ap().rearrange("(t p) n -> p t n", p=128)).then_inc(init, 16)
            eng.dma_start(
                w1.ap().rearrange("p (t n) -> p t n", t=16),
                w1_d.ap().rearrange("(t p) n -> p t n", p=128)).then_inc(init, 16)
            eng.dma_start(
                wfc.ap().rearrange("p (t n) -> p t n", t=8),
                wfc_d.ap().rearrange("(t p) n -> p t n", p=128)).then_inc(init, 16)
            eng.dma_start(b0.ap(), b0_d.ap()).then_inc(init, 16)
            eng.dma_start(b1.ap(), b1_d.ap()).then_inc(init, 16)
            eng.dma_start(bfc.ap(), bfc_d.ap()).then_inc(init, 16)
            eng.dma_start(ones.ap(), ones_d.ap()).then_inc(init, 16)
            eng.dma_start(
                h0T[1].ap().rearrange("p (t n) -> p t n", t=8),
                h0i_d.ap().rearrange("(t p) n -> p t n", p=128)).then_inc(init, 16)
            eng.dma_start(
                h1T[1].ap().rearrange("p (t n) -> p t n", t=8),
                h1i_d.ap().rearrange("(t p) n -> p t n", p=128)).then_inc(init, 16)
            eng.dma_start(cst[0].ap(), c0i_d.ap()).then_inc(init, 16)
            eng.dma_start(cst[1].ap(), c1i_d.ap()).then_inc(init, 16)
            eng.dma_start(
                xT[1].ap().rearrange("p (t n) -> p t n", t=8),
                yi_d.ap().rearrange("(t p) n -> p t n", p=128)).then_inc(init, 16)
            for t in range(seq):
                eng.wait_ge(act_y, t + 1)
                eng.wait_ge(rs_y[t % 2], 14 * fills(t))
                eng.dma_start(
                    out_d.ap()[t],
                    xT[t % 2].ap()[:, 0:256]).then_inc(dsem, 16)

        @block.tensor
        def _(eng):
            eng.wait_ge(init, INIT_V)

            def prefill_l0(hbuf):
                # bias + hh-part of next step's layer-0 gates; one
                # accumulation group per bank (start on the first mm)
                for g in range(4):
                    nc.tensor.matmul(greg(ps_g0a, ps_g0b, g),
                                     b0.ap()[:, 128 * g:128 * (g + 1)],
                                     ones.ap(), start=(g in (0, 3)),
                                     stop=False)
                for g in range(4):
                    for k in range(8):
                        nc.tensor.matmul(greg(ps_g0a, ps_g0b, g),
                                         wt(w0h, k, g), htile(hbuf, k),
                                         start=False, stop=False)

            prefill_l0(h0T[1])
            for t in range(seq):
                p, q = t % 2, (t + 1) % 2
                # ---- layer 0 gates: close with the x-part ----
                if t >= 1:
                    eng.wait_ge(act_y, t)        # own y chunk(t-1) in xT[q]
                    eng.wait_ge(rs_y[q], 14 * fills(t - 1))  # peers' chunks
                for g in range(3):
                    for k in range(4):
                        mm = nc.tensor.matmul(greg(ps_g0a, ps_g0b, g),
                                              wtx(k, g), htile(xT[q], k),
                                              start=False,
                                              stop=(g == 2 and k == 3))
                mm.then_inc(pe_g0a, 1)
                for k in range(4):
                    mm = nc.tensor.matmul(greg(ps_g0a, ps_g0b, 3),
                                          wtx(k, 3), htile(xT[q], k),
                                          start=False, stop=(k == 3))
                mm.then_inc(pe_g0b, 1)
                # ---- layer 1: bias + hh-part (overlaps L0's cell chain) ----
                if t >= 2:
                    eng.wait_ge(act_tg1, t - 1)  # ACT(t-2) done with ps_g1
                if t >= 1:
                    eng.wait_ge(dv_h1, t)
                    eng.wait_ge(rs_h1[q], 14 * fills(t - 1))
                for g in range(4):
                    nc.tensor.matmul(greg(ps_g1a, ps_g1b, g),
                                     b1.ap()[:, 128 * g:128 * (g + 1)],
                                     ones.ap(), start=(g in (0, 3)),
                                     stop=False)
                for g in range(4):
                    for k in range(8):
                        nc.tensor.matmul(greg(ps_g1a, ps_g1b, g),
                                         wt(w1, 8 + k, g), htile(h1T[q], k),
                                         start=False, stop=False)
                # ---- layer 1 ih-part: needs gathered h0(t) ----
                eng.wait_ge(dv_h0, t + 1)
                eng.wait_ge(rs_h0[p], 14 * fills(t))
                for g in range(3):
                    for k in range(8):
                        mm = nc.tensor.matmul(greg(ps_g1a, ps_g1b, g),
                                              wt(w1, k, g), htile(h0T[p], k),
                                              start=False,
                                              stop=(g == 2 and k == 7))
                mm.then_inc(pe_g1a, 1)
                for k in range(8):
                    mm = nc.tensor.matmul(greg(ps_g1a, ps_g1b, 3),
                                          wt(w1, k, 3), htile(h0T[p], k),
                                          start=False, stop=(k == 7))
                mm.then_inc(pe_g1b, 1)
                # ---- L0 prefill for t+1 (fills the h1-exchange window) ----
                if t + 1 < seq:
                    eng.wait_ge(act_tg0, t + 1)  # ACT(t) done with ps_g0
                    prefill_l0(h0T[p])
                # ---- fc (output-sharded: this core owns chunk pid%4) ----
                eng.wait_ge(dv_h1, t + 1)
                eng.wait_ge(rs_h1[p], 14 * fills(t))
                if t >= 1:
                    eng.wait_ge(act_y, t)        # relu(t-1) done with ps_y
                for k in range(8):
                    mm = nc.tensor.matmul(
                        ps_y.ap(),
                        wfc.ap()[:, 128 * k:128 * (k + 1)],
                        htile(h1T[p], k),
                        start=(k == 0), stop=(k == 7))
                mm.then_inc(pe_y, 1)

        @block.scalar
        def _(eng):
            eng.wait_ge(init, INIT_V)
            act_off = eng.partition_id() * 64
            for t in range(seq):
                p = t % 2
                for l, (psa, psb, sem_a, sem_b, s_s, s_tg, s_tc, dcs) in (
                        (0, (ps_g0a, ps_g0b, pe_g0a, pe_g0b,
                             act_s0, act_tg0, act_tc0, dv_c0)),
                        (1, (ps_g1a, ps_g1b, pe_g1a, pe_g1b,
                             act_s1, act_tg1, act_tc1, dv_c1))):
                    eng.wait_ge(sem_a, t + 1)
                    nc.scalar.activation(sbg[l].ap(), psa.ap(),
                                         AF.Sigmoid).then_inc(s_s, 1)
                    eng.wait_ge(sem_b, t + 1)
                    nc.scalar.activation(tg[l].ap(), psb.ap(),
                                         AF.Tanh).then_inc(s_tg, 1)
                    eng.wait_ge(dcs, t + 1)
                    nc.scalar.activation(th[l].ap(), cst[l].ap(),
                                         AF.Tanh).then_inc(s_tc, 1)
                # fc relu -> own slot of xT[p] (also the y-exchange source)
                eng.wait_ge(pe_y, t + 1)
                if t >= 1:
                    eng.wait_ge(dsem, 16 * t)    # out-DMA(t-1) done
                if t >= 2:
                    eng.wait_ge(lsy, 16 * (t - 1))   # y-bcast(t-2) drained
                nc.scalar.activation(xT[p].ap()[:, bass.ds(act_off, 64)],
                                     ps_y.ap(), AF.Relu,
                                     bias=bfc.ap()).then_inc(act_y, 1)

        @block.vector
        def _(eng):
            eng.wait_ge(init, INIT_V)
            dv_off = eng.partition_id() * 64
            for t in range(seq):
                p = t % 2
                for l, (s_s, s_tg, s_tc, dcs, dhs, hbuf, lsem) in (
                        (0, (act_s0, act_tg0, act_tc0, dv_c0, dv_h0,
                             h0T[p], ls0)),
                        (1, (act_s1, act_tg1, act_tc1, dv_c1, dv_h1,
                             h1T[p], ls1))):
                    c_ap = cst[l].ap()
                    eng.wait_ge(s_s, t + 1)
                    nc.vector.tensor_tensor(tmp[l][0].ap(),
                                            sbg[l].ap()[:, 0:64], c_ap,
                                            ALU.mult)          # f * c
                    eng.wait_ge(s_tg, t + 1)
                    nc.vector.tensor_tensor(tmp[l][1].ap(),
                                            sbg[l].ap()[:, 64:128],
                                            tg[l].ap(),
                                            ALU.mult)          # i * tanh(g)
                    if t >= 1:
                        eng.wait_ge(s_tc, t)   # tanh(c(t-1)) read done
                    nc.vector.tensor_tensor(c_ap, tmp[l][0].ap(),
                                            tmp[l][1].ap(),
                                            ALU.add).then_inc(dcs, 1)
                    eng.wait_ge(s_tc, t + 1)
                    if t >= 2:
                        eng.wait_ge(lsem, 16 * (t - 1))  # bcast(t-2) drained
                    nc.vector.tensor_tensor(hbuf.ap()[:, bass.ds(dv_off, 64)],
                                            sbg[l].ap()[:, 128:192],
                                            th[l].ap(),
                                            ALU.mult).then_inc(dhs, 1)

        @block.gpsimd
        def _(eng):
            eng.wait_ge(init, INIT_V)
            gp_off = eng.partition_id() * 64
            rdests = [None] + [(0, d ^ 2) if d >= 4 else (0, d)
                               for d in range(1, 8)]

            def stage(t):
                p = t % 2
                for buf, rsem, lsem in ((h0T[p], rs_h0[p], ls0),
                                        (h1T[p], rs_h1[p], ls1),
                                        (xT[p], rs_y[p], lsy)):
                    slot = buf.ap()[:, bass.ds(gp_off, 64)]
                    eng.remote_dma_broadcast(
                        slot, slot, remote_sem=rsem, local_sem=lsem,
                        rdests=rdests).then_inc(prep, 1)

            stage(0)
            for t in range(seq):
                eng.wait_ge(prep, 3 * t + 1)
                eng.wait_ge(dv_h0, t + 1)
                eng.trigger_dma(count=1)
                eng.wait_ge(prep, 3 * t + 2)
                eng.wait_ge(dv_h1, t + 1)
                eng.trigger_dma(count=1)
                eng.wait_ge(prep, 3 * t + 3)
                eng.wait_ge(act_y, t + 1)
                eng.trigger_dma(count=1)
                if t + 1 < seq:
                    stage(t + 1)

    nc.compile()
    return nc


def _prep_inputs(core, W_ih0, W_hh0, b_ih0, b_hh0, W_ih1, W_hh1, b_ih1, b_hh1,
                 W_fc, b_fc, h0, c0):
    c = core
    # gate-chunk order (f, i, o, g); PyTorch row order is (i, f, g, o)
    rows = np.concatenate([np.arange(g * H + c * HPC, g * H + (c + 1) * HPC)
                           for g in (1, 0, 3, 2)])
    import ml_dtypes
    f = np.float32
    bf = ml_dtypes.bfloat16
    w1 = np.concatenate([W_ih1[rows].T, W_hh1[rows].T], axis=0)
    return {
        "w0x": np.ascontiguousarray(W_ih0[rows].T).astype(bf),
        "w0h": np.ascontiguousarray(W_hh0[rows].T).astype(bf),
        "w1": np.ascontiguousarray(w1).astype(bf),
        "wfc": np.ascontiguousarray(
            W_fc.T[:, 128 * (c % 4):128 * (c % 4 + 1)]).astype(bf),
        "b0": np.ascontiguousarray((b_ih0 + b_hh0)[rows][None, :]).astype(bf),
        "b1": np.ascontiguousarray((b_ih1 + b_hh1)[rows][None, :]).astype(bf),
        "bfc": np.ascontiguousarray(
            b_fc[128 * (c % 4):128 * (c % 4 + 1), None], f),
        "ones": np.ones((1, 64), bf),
        "h0i": np.ascontiguousarray(h0[0].T).astype(bf),
        "h1i": np.ascontiguousarray(h0[1].T).astype(bf),
        "c0i": np.ascontiguousarray(c0[0][:, c * HPC:(c + 1) * HPC].T, f),
        "c1i": np.ascontiguousarray(c0[1][:, c * HPC:(c + 1) * HPC].T, f),
        "yi": np.full((1024, 64), SOS_VALUE, bf),
    }


def run(seq, in_maps, trace=False, trace_kwargs=None):
    from concourse.bass_utils import run_bass_kernel_spmd

    key = int(seq)
    if key not in _CACHE:
        _CACHE[key] = _build(key)
    nc = _CACHE[key]
    kw = {}
    if trace:
        kw = dict(trace=True, trace_cores=[0], **(trace_kwargs or {}))
    return run_bass_kernel_spmd(nc, in_maps, core_ids=list(range(N_CORES)),
                                **kw)


def kernel(encoder_output=None, h0=None, c0=None, W_ih0=None, W_hh0=None,
           b_ih0=None, b_hh0=None, W_ih1=None, W_hh1=None, b_ih1=None,
           b_hh1=None, W_fc=None, b_fc=None, seq_length=256, _trace=False):
    seq = int(seq_length)
    args = (W_ih0, W_hh0, b_ih0, b_hh0, W_ih1, W_hh1, b_ih1, b_hh1, W_fc, b_fc,
            h0, c0)
    args = tuple(np.asarray(a, np.float32) for a in args)
    in_maps = [_prep_inputs(c, *args) for c in range(N_CORES)]
    res = run(seq, in_maps, trace=_trace)
    out = np.asarray(res.results[0]["out"]).astype(np.float32)
    y = out.reshape(seq, 128, 4, 64).transpose(3, 0, 2, 1).reshape(B, seq, OUT)
    if _trace:
        kernel._last_results = res
    return np.ascontiguousarray(y)

